# revision 17
# baseline (speedup 1.0000x reference)
"""Head-parallel MultiHeadAttention kernel for 8 Trainium2 NeuronCores.

Original AllReduce variant with bf16 default (measured 415571 ns,
L2 err 1.68e-3). Kept as the safe fallback.
"""
import os
import sys

sys.path.insert(0, "/opt/trn_rl_repo")
sys.path.insert(0, "/root/.axon_site")

import numpy as np

import concourse.bacc as bacc
import concourse.mybir as mybir
from concourse.tile import TileContext
from concourse import bass_utils

P = 128
B, S, D, H = 2, 2048, 512, 8
NCORES = 8
DT = D // P
MC = S // 512
KT = S // P
F32 = mybir.dt.float32
F32R = mybir.dt.float32r
BF16 = mybir.dt.bfloat16
FP8 = mybir.dt.float8e4
DR = mybir.MatmulPerfMode.DoubleRow

MM_DTYPE = os.environ.get("KERNEL_DTYPE", "bf16")

_NC_CACHE = {}


def _build_nc(mm_dtype):
    MMD = F32R if mm_dtype == "f32r" else BF16
    IND = F32 if mm_dtype == "f32r" else BF16
    big_bufs = 1 if mm_dtype == "f32r" else 2

    nc = bacc.Bacc("TRN2", target_bir_lowering=False, debug=False,
                   num_devices=NCORES)

    qT = nc.dram_tensor("qT", [B, D, S], IND, kind="ExternalInput")
    kHi = nc.dram_tensor("kHi", [B, D, S], FP8, kind="ExternalInput")
    kLo = nc.dram_tensor("kLo", [B, D, S], FP8, kind="ExternalInput")
    ckd = nc.dram_tensor("ck", [B, S], F32, kind="ExternalInput")
    vn = nc.dram_tensor("vn", [B, S, D], IND, kind="ExternalInput")
    wm = nc.dram_tensor("wm", [D, D], IND, kind="ExternalInput")
    w2 = nc.dram_tensor("w2", [D, D], IND, kind="ExternalInput")
    onesinv = nc.dram_tensor("onesinv", [P, 2], IND, kind="ExternalInput")
    out = nc.dram_tensor("out", [B, S, D], F32, kind="ExternalOutput")

    ar_out = [
        nc.dram_tensor(f"ar_out{b}_{qc}", [512, D], F32, addr_space="Shared")
        for b in range(B) for qc in range(MC)
    ]

    def cast_mm(ap):
        return ap.bitcast(F32R) if mm_dtype == "f32r" else ap

    with TileContext(nc) as tc:
        with (
            tc.tile_pool(name="consts", bufs=1) as consts,
            tc.tile_pool(name="qts", bufs=2) as qts,
            tc.tile_pool(name="big", bufs=big_bufs) as big,
            tc.tile_pool(name="pts", bufs=1) as pts,
            tc.tile_pool(name="small", bufs=3) as small,
            tc.tile_pool(name="ostage", bufs=3) as ostage,
            tc.tile_pool(name="rot", bufs=3, space="PSUM") as rot,
            tc.tile_pool(name="psout", bufs=1, space="PSUM") as psout,
            tc.tile_pool(name="dram", bufs=1, space="DRAM") as dram,
        ):
            def load_w(t):
                w_sb = consts.tile([P, DT, D], MMD, name=t.name + "_sb")
                nc.sync.dma_start(
                    w_sb[:],
                    cast_mm(t[:].rearrange("(dt p) e -> p dt e", p=P)),
                )
                return w_sb

            wm_sb = consts.tile([P, DT, D], MMD, name="wm_sb")
            wm_ap = wm[:].rearrange("(dt p) e -> p dt e", p=P)
            nc.sync.dma_start(wm_sb[:, :, 0:P], cast_mm(wm_ap[:, :, 0:P]))
            ck_sb = consts.tile([P, B, KT], F32, name="ck_sb")
            for _b in range(B):
                nc.sync.dma_start(ck_sb[:, _b, :],
                                  ckd[_b].rearrange("(kt p) -> p kt", p=P))

            def load_act(dst, src_b_ap, piecewise=True):
                if piecewise:
                    for c in range(MC):
                        csl = slice(c * 512, (c + 1) * 512)
                        nc.sync.dma_start(dst[:, :, csl],
                                          cast_mm(src_b_ap[:, :, csl]))
                else:
                    nc.sync.dma_start(dst[:], cast_mm(src_b_ap))

            for _e in range(1, DT):
                nc.sync.dma_start(wm_sb[:, :, _e * P:(_e + 1) * P],
                                  cast_mm(wm_ap[:, :, _e * P:(_e + 1) * P]))

            def alloc_acts(b):
                return (big.tile([P, DT, S], MMD, tag="QRAW", name=f"q{b}"),
                        big.tile([P, DT, S // 2], FP8, tag="KHL", name=f"khl{b}"),
                        big.tile([P, DT, S // 2], FP8, tag="KHH", name=f"khh{b}"),
                        big.tile([P, DT, S // 2], FP8, tag="KLL", name=f"kll{b}"),
                        big.tile([P, DT, S // 2], FP8, tag="KLH", name=f"klh{b}"),
                        big.tile([P, KT // 2, D], MMD, tag="VLO", name=f"vlo{b}"),
                        big.tile([P, KT // 2, D], MMD, tag="VHI", name=f"vhi{b}"))

            def load_kv(b, a):
                khap = kHi[b].rearrange("(dt p) s -> p dt s", p=P)
                klap = kLo[b].rearrange("(dt p) s -> p dt s", p=P)
                vap = vn[b].rearrange("(kt p) d -> p kt d", p=P)
                for half in range(2):
                    ksl = slice(half * 1024, (half + 1) * 1024)
                    nc.sync.dma_start(a[1 + half][:], khap[:, :, ksl])
                    nc.sync.dma_start(a[3 + half][:], klap[:, :, ksl])
                for half in range(2):
                    vsl = slice(half * 8, (half + 1) * 8)
                    nc.sync.dma_start(a[5 + half][:], cast_mm(vap[:, vsl, :]))

            acts = {}
            acts[0] = alloc_acts(0)
            q0ap = qT[0].rearrange("(dt p) s -> p dt s", p=P)
            nc.sync.dma_start(acts[0][0][:, :, 0:512], cast_mm(q0ap[:, :, 0:512]))
            load_kv(0, acts[0])
            for c in range(1, MC):
                csl = slice(c * 512, (c + 1) * 512)
                nc.sync.dma_start(acts[0][0][:, :, csl], cast_mm(q0ap[:, :, csl]))
            oinv_sb = consts.tile([P, 2], MMD, name="oinv_sb")
            nc.sync.dma_start(oinv_sb[:], cast_mm(onesinv[:]))
            w2_sb = load_w(w2)
            if big_bufs >= 2:
                acts[1] = alloc_acts(1)
                load_act(acts[1][0], qT[1].rearrange("(dt p) s -> p dt s", p=P))
                load_kv(1, acts[1])

            partial = [
                dram.tile([512, D], F32, name=f"partial{b}_{qc}")
                for b in range(B) for qc in range(MC)
            ]

            for b in range(B):
                if b > 0 and b not in acts:
                    acts[b] = alloc_acts(b)
                    load_act(acts[b][0],
                             qT[b].rearrange("(dt p) s -> p dt s", p=P))
                    load_kv(b, acts[b])
                q_full = acts[b][0]

                def k_pair(src_idx, dtp, kt, _a=acts[b]):
                    # [128, 2, 128] fp8: dt-pair x key-block for DoubleRow
                    t = _a[src_idx + (0 if kt < 8 else 1)]
                    kk = kt % 8
                    return t[:, 2 * dtp:2 * dtp + 2, kk * P:(kk + 1) * P]

                def v_tile(kt, et, _a=acts[b]):
                    t = _a[5] if kt < 8 else _a[6]
                    return t[:, kt % 8, et * P:(et + 1) * P]

                for qc in range(MC):
                    qsl = slice(qc * 512, (qc + 1) * 512)
                    QTc = qts.tile([P, DT, 512], FP8, tag="QT")
                    for et in range(DT):
                        ps = rot.tile([P, 512], F32, tag="ps")
                        for dt in range(DT):
                            nc.tensor.matmul(
                                ps[:],
                                lhsT=wm_sb[:, dt, et * P:(et + 1) * P],
                                rhs=q_full[:, dt, qsl],
                                start=(dt == 0), stop=(dt == DT - 1),
                            )
                        nc.scalar.activation(
                            QTc[:, et, :], ps[:],
                            mybir.ActivationFunctionType.Copy,
                            scale=64.0,
                        )
                    PT = pts.tile([P, KT, 512], MMD, tag="PT")
                    for kt in range(KT):
                        ps = rot.tile([P, 512], F32, tag="ps")
                        step = 0
                        for dtp in range(2):
                            for src_idx in (1, 3):  # k_hi, k_lo
                                nc.tensor.matmul(
                                    ps[:],
                                    lhsT=k_pair(src_idx, dtp, kt),
                                    rhs=QTc[:, 2 * dtp:2 * dtp + 2, :],
                                    perf_mode=DR,
                                    start=(step == 0), stop=(step == 3),
                                )
                                step += 1
                        nc.scalar.activation(
                            PT[:, kt, :], ps[:],
                            mybir.ActivationFunctionType.Exp,
                            bias=ck_sb[:, b, kt:kt + 1],
                            scale=0.015625,
                        )
                    outT_ps = psout.tile([P, DT, 512], F32, tag="outT")
                    for kt in range(KT):
                        for et in range(DT):
                            nc.tensor.matmul(
                                outT_ps[:, et, :],
                                lhsT=v_tile(kt, et),
                                rhs=PT[:, kt, :],
                                start=(kt == 0), stop=(kt == KT - 1),
                            )
                    def ptf(kt):
                        ap = PT[:, kt, :]
                        return ap.bitcast(F32) if mm_dtype == "f32r" else ap
                    denAcc = small.tile([P, 512], F32, tag="denAcc")
                    nc.vector.tensor_add(denAcc[:], ptf(0), ptf(1))
                    for kt in range(2, KT):
                        nc.vector.tensor_add(denAcc[:], denAcc[:], ptf(kt))
                    denB_sb = small.tile([P, 512], MMD, tag="denB_sb")
                    nc.vector.tensor_copy(denB_sb[:], denAcc[:])
                    denT_ps = rot.tile([P, 512], F32, tag="ps")
                    for t in range(4):
                        nc.tensor.matmul(
                            denT_ps[:, 2 * t:2 * t + 2],
                            lhsT=denB_sb[:, t * P:(t + 1) * P],
                            rhs=oinv_sb[:],
                            start=True, stop=True,
                        )
                    recipT = small.tile([P, 8], F32, tag="recipT")
                    nc.vector.reciprocal(recipT[:], denT_ps[:, 0:8])
                    AT_sb = small.tile([P, DT, 512], MMD, tag="AT")
                    for et in range(DT):
                        nc.vector.tensor_copy(AT_sb[:, et, :], outT_ps[:, et, :])
                    pidx = b * MC + qc
                    for t in range(4):
                        ps = rot.tile([P, 512], F32, tag="ps")
                        for et in range(DT):
                            nc.tensor.matmul(
                                ps[:],
                                lhsT=AT_sb[:, et, t * P:(t + 1) * P],
                                rhs=w2_sb[:, et, :],
                                start=(et == 0), stop=(et == DT - 1),
                            )
                        o_sb = ostage.tile([P, 512], F32, tag="o")
                        nc.vector.tensor_scalar_mul(
                            o_sb[:], ps[:], recipT[:, 2 * t:2 * t + 1]
                        )
                        nc.sync.dma_start(partial[pidx][t * P:(t + 1) * P, :],
                                          o_sb[:])

                    nc.gpsimd.collective_compute(
                        "AllReduce",
                        mybir.AluOpType.add,
                        replica_groups=[list(range(NCORES))],
                        ins=[partial[pidx][:].opt()],
                        outs=[ar_out[pidx][:].opt()],
                    )
                    nc.gpsimd.dma_start(
                        out[b, qc * 512:(qc + 1) * 512, :], ar_out[pidx][:]
                    )

    nc.compile()
    return nc


def kernel(q, k, v, Wq, Wk, Wv, bq, bk, bv, Wo, bo):
    key = ("nc", MM_DTYPE)
    if key not in _NC_CACHE:
        _NC_CACHE[key] = _build_nc(MM_DTYPE)
    nc = _NC_CACHE[key]

    q = np.asarray(q, dtype=np.float32)
    k = np.asarray(k, dtype=np.float32)
    v = np.asarray(v, dtype=np.float32)
    Wq = np.asarray(Wq, dtype=np.float32)
    Wk = np.asarray(Wk, dtype=np.float32)
    Wv = np.asarray(Wv, dtype=np.float32)
    bq = np.asarray(bq, dtype=np.float32)
    bv = np.asarray(bv, dtype=np.float32)
    Wo = np.asarray(Wo, dtype=np.float32)
    bo = np.asarray(bo, dtype=np.float32)

    import ml_dtypes

    def cast(x):
        return np.ascontiguousarray(
            np.asarray(x, dtype=np.float32).astype(ml_dtypes.bfloat16))

    scale = np.float32(1.0 / np.sqrt(D))
    qT = cast(q.transpose(0, 2, 1))
    kTf = np.ascontiguousarray(k.transpose(0, 2, 1))
    k_hi8 = kTf.astype(ml_dtypes.float8_e4m3)
    k_lo8 = (kTf - k_hi8.astype(np.float32)).astype(ml_dtypes.float8_e4m3)
    vn = cast(v)
    onesinv = cast(np.ones((P, 2), dtype=np.float32))

    in_maps = []
    for h in range(NCORES):
        Wo_h = Wo[h * D:(h + 1) * D, :]
        u_h = (bq[h] * scale) @ Wk[h].T
        in_maps.append({
            "qT": qT, "kHi": k_hi8, "kLo": k_lo8, "vn": vn,
            "wm": cast((Wq[h] * scale) @ Wk[h].T),
            "w2": cast(Wv[h] @ Wo_h),
            "ck": np.ascontiguousarray(np.einsum("bsd,d->bs", k, u_h,
                                                 dtype=np.float32)),
            "onesinv": onesinv,
        })

    trace = bool(int(os.environ.get("KERNEL_TRACE", "0")))
    if trace:
        try:
            import trace_hook
            trace_hook.install()
        except Exception:
            pass
    res = bass_utils.run_bass_kernel_spmd(
        nc, in_maps, core_ids=list(range(NCORES)), trace=trace
    )
    _NC_CACHE["last_result"] = res

    out = np.array(res.results[0]["out"])  # [B, S, D]
    c_const = sum(bv[h] @ Wo[h * D:(h + 1) * D, :] for h in range(H)) + bo
    out += c_const[None, None, :].astype(np.float32)
    return out.astype(np.float32)
